# revision 3
# baseline (speedup 1.0000x reference)
"""Cost-volume kernel (nn_CostVolume) for Trainium2, 8 NeuronCores.

out[b, i, h, w] = mean_c feat1[b, c, h, w] * feat2[b, c, h, w + i - 4]
(feat2 zero-padded along width), inputs (8, 256, 96, 320) fp32,
output (8, 9, 96, 320) fp32.  Data-parallel over B: core b handles
batch b (communication-free).

Per core the 9 shifted channel-dot-products are computed as banded
correlation matmuls band[p, n] = sum_c f1[c, w0+p] * f2[c, w0-4+n] on
the TensorEngine, with the C=256 contraction split into two
PSUM-accumulated K=128 matmuls.  Diagonals band[p, p+i] (inexpressible
by lockstep engines) are extracted via a bf16 HBM scratch round-trip:
bands for a 32-row h-slice are dumped in (w-block, n, h) layout, then
one strided DMA per w-block gathers columns [p, p+8] as 9*32-element
runs; PE transposes assemble [h, w] output tiles.

bf16 datapath (vs the fp32 V1 at ~131 us):
- Inputs are cast fp32->bf16 *during* the HBM->SBUF load via gpsimd
  (SWDGE) casting DMAs — zero vector-engine cast cost.
- bf16 matmuls run 1 cycle/row vs fp32's 4 (cost model
  instruction_cost_v2.rs), cutting PE busy ~115us -> ~29us/core, which
  moves the kernel from PE-bound to input-DMA-bound (63 MB/core fp32).
- The scratch round-trip is bf16 (5 MB vs 10 MB); transposes bf16.
- psum->osb copies cast back to fp32 (rel err 3.0e-3 < 2e-2).

Measured (reps-slope, axon-tunneled trn2): ~70-105 us per rep vs
V1's ~111-140 us in the same harness (~25% faster; relay jitter
dominates the spread).  Rejected variants: grouped 16-column dumps
(+40 us — narrow 8-partition DMAs lose SDMA parallelism) and NH=16
input chunks (slightly slower — worse pipeline granularity)."""

import numpy as np

import concourse.bacc as bacc
import concourse.bass as bass
import concourse.tile as tile
from concourse import mybir
from concourse.bass_utils import run_bass_kernel_spmd
from concourse.masks import make_identity

B, C, H, W = 8, 256, 96, 320
D = 4
NS = 2 * D + 1  # 9 shifts
P = 128  # partitions per c-block
M = 64  # w-block size
NB = W // M  # 5 w-blocks
NBAND = M + 2 * D  # 72 band columns
NH = 8  # h rows per feature chunk
NCHUNK = H // NH  # 24
WP = W + 2 * D  # padded feat2 row
PS_BUFS = 4  # psum banks for matmul accumulation
TP_BUFS = 4  # psum banks for tail transposes
NHALF = 3  # image slices for tail pipelining
H2 = H // NHALF  # h rows per slice (32)
CPH = NCHUNK // NHALF  # chunks per slice

F32 = mybir.dt.float32
BF16 = mybir.dt.bfloat16

_cache: dict = {}


def _build(reps: int = 1, skip_gather: bool = False, skip_compute: bool = False,
           skip_mm: bool = False, skip_act: bool = False):
    nc = bacc.Bacc("TRN2", target_bir_lowering=False, debug=False, num_devices=B)
    f1 = nc.dram_tensor("f1", (C, H, W), F32, kind="ExternalInput")
    f2 = nc.dram_tensor("f2", (C, H, W), F32, kind="ExternalInput")
    out = nc.dram_tensor("out", (NS, H, W), F32, kind="ExternalOutput")

    with tile.TileContext(nc) as tc:
        with (
            tc.tile_pool(name="consts", bufs=1) as cpool,
            tc.tile_pool(name="feat", bufs=2) as fpool,
            tc.tile_pool(name="band", bufs=1) as bpool,
            tc.tile_pool(name="gat", bufs=4) as gpool,
            tc.tile_pool(name="osb", bufs=3) as opool,
            tc.tile_pool(name="ps", bufs=PS_BUFS, space="PSUM") as pspool,
            tc.tile_pool(name="scratch", bufs=1, space="DRAM") as dpool,
        ):
            ident = cpool.tile([M, M], BF16)
            make_identity(nc, ident)
            pools = (fpool, bpool, gpool, opool, pspool, dpool)
            for _rep in range(reps):
                _body(
                    nc, tc, pools, ident, f1, f2, out,
                    skip_gather=skip_gather, skip_compute=skip_compute,
                    skip_mm=skip_mm, skip_act=skip_act,
                )

    nc.compile()
    return nc


def _tail(nc, pools, ident, out, band, half):
    """Dump one slice's bands to HBM scratch (bf16), gather diagonals,
    transpose, and write out[:, half*H2:(half+1)*H2, :] (fp32)."""
    fpool, bpool, gpool, opool, pspool, dpool = pools

    scratch = dpool.tile([M, NB, NBAND, H2], BF16, tag=f"scr{half % 2}")
    for blk in range(NB):
        nc.sync.dma_start(
            out=scratch[:, blk].rearrange("p n h -> p (n h)"),
            in_=band[:, blk].rearrange("p n h -> p (n h)"),
        )

    # Diagonals of row p are columns [p, p+8]: with h innermost these are
    # 9*H2 consecutive scratch elements per partition-row, so one DMA per
    # w-block gathers all shifts with (row+1)-strided 9*H2-element runs.
    sc_p = NB * NBAND * H2  # scratch partition-row length in elements
    g9s = []
    with nc.allow_non_contiguous_dma("banded diagonal gather"):
        for blk in range(NB):
            g9 = gpool.tile([M, NS, H2], BF16, tag=f"g9_{blk}", bufs=1)
            src = bass.AP(
                tensor=scratch.tensor,
                offset=scratch.offset + blk * NBAND * H2,
                ap=[[sc_p + H2, M], [1, NS * H2]],
            )
            nc.sync.dma_start(out=g9.rearrange("p i h -> p (i h)"), in_=src)
            g9s.append(g9)
    # Transpose shift-triples [64, 3*H2] -> [3*H2, 64] in one PE op each,
    # splitting the psum->osb (bf16 -> fp32) copies across ACT and DVE.
    for it in range(NS // 3):
        osbs = []
        for k in range(3):
            osb = opool.tile(
                [H2, W], F32, tag=f"osb{k}", bufs=2, name=f"osb_{it}_{k}"
            )
            osbs.append(osb)
        for blk in range(NB):
            tp = pspool.tile([3 * H2, M], BF16, tag="tp", bufs=TP_BUFS)
            nc.tensor.transpose(
                tp, g9s[blk][:, 3 * it : 3 * it + 3, :].rearrange("p a b -> p (a b)"),
                ident,
            )
            for k in range(3):
                dst = osbs[k][:, blk * M : (blk + 1) * M]
                srcp = tp[k * H2 : (k + 1) * H2, :]
                if (blk + k) % 2 == 0:
                    nc.scalar.copy(out=dst, in_=srcp)
                else:
                    nc.vector.tensor_copy(dst, srcp)
        for k in range(3):
            i = 3 * it + k
            nc.scalar.dma_start(
                out=out.ap()[i, half * H2 : (half + 1) * H2, :], in_=osbs[k]
            )


def _body(nc, tc, pools, ident, f1, f2, out,
          skip_gather=False, skip_compute=False, skip_mm=False, skip_act=False):
    fpool, bpool, gpool, opool, pspool, dpool = pools

    # Pre-allocated, manually double-buffered bf16 f2 tiles: the D-wide zero
    # pads are written once; chunk DMAs only touch the [D, D+W) interior.
    f2slots = [
        [
            fpool.tile(
                [P, NH, WP], BF16, tag=f"f2_{cb}_{j}", bufs=1, name=f"f2s_{cb}_{j}"
            )
            for j in range(2)
        ]
        for cb in range(2)
    ]
    for cb in range(2):
        for j in range(2):
            nc.vector.memset(f2slots[cb][j][:, :, 0:D], 0.0)
            nc.vector.memset(f2slots[cb][j][:, :, D + W : WP], 0.0)

    for half in range(NHALF):
        # SBUF-resident bands for this slice: [p, blk, n, h2], bf16.
        band = bpool.tile([M, NB, NBAND, H2], BF16, tag=f"band{half % 2}")

        for chunk in range(CPH):
            h0 = half * H2 + chunk * NH
            f1t = []
            f2t = []
            for cb in range(2):
                t1 = fpool.tile([P, NH, W], BF16, tag=f"f1_{cb}")
                nc.gpsimd.dma_start(
                    out=t1, in_=f1.ap()[cb * P : (cb + 1) * P, h0 : h0 + NH, :]
                )
                f1t.append(t1)
                t2 = f2slots[cb][(half * CPH + chunk) % 2]
                nc.gpsimd.dma_start(
                    out=t2[:, :, D : D + W],
                    in_=f2.ap()[cb * P : (cb + 1) * P, h0 : h0 + NH, :],
                )
                f2t.append(t2)

            if skip_compute:
                continue
            for hl in range(NH):
                hloc = chunk * NH + hl  # h index within this slice
                ps = pspool.tile([M, NB * NBAND], F32, tag="ps")
                if not skip_mm:
                    for blk in range(NB):
                        w0 = blk * M
                        for cb in range(2):
                            nc.tensor.matmul(
                                ps[:, blk * NBAND : (blk + 1) * NBAND],
                                f1t[cb][:, hl, w0 : w0 + M],
                                f2t[cb][:, hl, w0 : w0 + NBAND],
                                start=(cb == 0),
                                stop=(cb == 1),
                            )
                if not skip_act:
                    # psum (blk, n) -> band[:, blk, n, hloc] (cast to bf16),
                    # alternating ACT/DVE so two engines split the copy wall.
                    if hl % 2 == 0:
                        nc.scalar.activation(
                            band[:, :, :, hloc],
                            ps.rearrange("p (b n) -> p b n", b=NB),
                            mybir.ActivationFunctionType.Copy,
                            scale=1.0 / C,
                        )
                    else:
                        nc.vector.tensor_scalar_mul(
                            band[:, :, :, hloc],
                            ps.rearrange("p (b n) -> p b n", b=NB),
                            1.0 / C,
                        )

        if skip_compute or skip_gather:
            continue
        _tail(nc, pools, ident, out, band, half)


def kernel(feat1: np.ndarray, feat2: np.ndarray) -> np.ndarray:
    assert feat1.shape == (B, C, H, W), feat1.shape
    assert feat2.shape == (B, C, H, W), feat2.shape
    if "nc" not in _cache:
        _cache["nc"] = _build()
    nc = _cache["nc"]
    feat1 = np.ascontiguousarray(feat1, dtype=np.float32)
    feat2 = np.ascontiguousarray(feat2, dtype=np.float32)
    in_maps = [{"f1": feat1[b], "f2": feat2[b]} for b in range(B)]
    res = run_bass_kernel_spmd(nc, in_maps, core_ids=list(range(B)))
    return np.stack([res.results[b]["out"] for b in range(B)], axis=0)
